# revision 11
# baseline (speedup 1.0000x reference)
"""AttentiveAggregation (segment softmax-pool) Trainium2 kernel.

Math (per graph g): out_g = sum_v alpha_v H_v,  alpha = softmax_g(e),
  e_v = w_score . tanh(W_proj @ H_v + b_proj).

Key transformations:
 * Global shift: softmax is shift invariant per segment, and
   |e| <= ||w_score||_1 (tanh bounded), so a single global constant
   C = ||w_score||_1 replaces the per-segment max. Then
   out_g = (sum_v a_v [H_v|1])[:D] / (...)[D]  with a_v = exp(e_v - C):
   two segment sums, done as one one-hot matmul with a ones-column.
 * Segment-aligned tiling: segments are grouped into blocks of S.
   batch is sorted, so each block's nodes are a contiguous range.  The
   host repacks H into per-(block, tile) arrays so every device access
   is static; tiles that spill into the next block contribute zero via
   the one-hot mask (node's local seg id falls outside 0..S-1).
 * Transposed projection: proj_T[h, node] = (sgn(w) W_proj) @ H_v keeps
   the feature dim on partitions, so b_proj rides the ACT tanh bias
   (per-partition) and the score dot e = |w| . tanh_T happens on the PE
   (contraction over partitions, N=1 matmul) instead of the DVE.
 * Sharding: 8 cores x NBLK blocks each; cores are fully independent
   (no collectives); host concatenates the per-core slabs.
"""

import math

import numpy as np

P = 128                    # partitions / tile node count / D / HS
D = 128
G_SEGS = 16384
N_CORES = 8

# alpha values are prescaled by exp(PRESCALE_LN) inside the exp bias so the
# smallest per-segment weights stay in f16 normal range; numerator and
# denominator scale together so the final division cancels it exactly.
PRESCALE_LN = 14 * math.log(2.0)

# dtypes: "f32" | "bf16" | "f16" | "f8"
CFG = {
    "ht": "f16",     # transposed H feeding the projection matmul
    "rhs": "f16",    # [H|1] feeding the segment-sum matmul (and one-hot)
    "act": "f16",    # tanh output / |w| moving operand
    "ht_scale": 1.0,   # host premultiplier on ht (undone by ACT scale)
    "wt_scale": 1.0,   # host premultiplier on wt (undone by ACT scale)
}

VARIANT = {
    "segs_per_blk": 128,   # segments per block (S); out tile is [S, D]
    "grp": 8,              # tiles per compute/PSUM group
    "dma_grp": 16,         # tiles per stream-DMA instruction
    "rhs_eng": "sync",     # engine issuing the rhs stream DMAs (HWDGE)
    "ht_eng": "gpsimd",    # engine issuing the ht stream DMAs (SWDGE)
    "bufs_io": 4,          # ht/rhs stream pools
    "bufs_mid": 6,         # intermediate tiles
    "bufs_pp": 2,          # projection PSUM pool
    "bufs_ep": 2,          # score PSUM pool
    "bufs_op": 2,          # output-accumulator PSUM pool
}

_NP_DT = {"f32": np.float32, "f16": np.float16}


def _np_dt(name):
    if name in ("bf16", "f8"):
        import ml_dtypes
        return {"bf16": ml_dtypes.bfloat16,
                "f8": ml_dtypes.float8_e4m3}[name]
    return _NP_DT[name]


def _my_dt(name, mybir):
    return {
        "f32": mybir.dt.float32,
        "bf16": mybir.dt.bfloat16,
        "f16": mybir.dt.float16,
        "f8": mybir.dt.float8e4,
    }[name]


def _build_program(t_max, c_shift, cfg, use_delta=True, repeat=1):
    import concourse.bacc as bacc
    import concourse.mybir as mybir
    import concourse.tile as tile

    f32 = mybir.dt.float32
    dt_ht = _my_dt(cfg["ht"], mybir)
    dt_rhs = _my_dt(cfg["rhs"], mybir)
    dt_act = _my_dt(cfg["act"], mybir)
    S = VARIANT["segs_per_blk"]
    GRP = VARIANT["grp"]
    DGRP = VARIANT["dma_grp"]
    NBLK = (G_SEGS // S) // N_CORES
    NGD = t_max // DGRP
    act_scale = 1.0 / (cfg["ht_scale"] * cfg["wt_scale"])

    nc = bacc.Bacc(None, target_bir_lowering=False)
    hrhs_d = nc.dram_tensor("hrhs", [NBLK, NGD, P, DGRP * (D + 1)], dt_rhs,
                            kind="ExternalInput")
    ht_d = nc.dram_tensor("ht", [NBLK, NGD, P, DGRP * P], dt_ht,
                          kind="ExternalInput")
    bl_d = nc.dram_tensor("bl", [NBLK, P, t_max], f32,
                          kind="ExternalInput")
    wt_d = nc.dram_tensor("wt", [P, D], dt_ht, kind="ExternalInput")
    wabs_d = nc.dram_tensor("wabs", [P, 1], dt_act, kind="ExternalInput")
    bcol_d = nc.dram_tensor("bcol", [P, 1], f32, kind="ExternalInput")
    iota_d = nc.dram_tensor("iota", [P, S], dt_rhs, kind="ExternalInput")
    out_d = nc.dram_tensor("out", [NBLK, S, D], f32, kind="ExternalOutput")

    with tile.TileContext(nc) as tc:
        with (
            tc.tile_pool(name="const", bufs=1) as constp,
            tc.tile_pool(name="htp", bufs=VARIANT["bufs_io"]) as htp,
            tc.tile_pool(name="rhsp", bufs=VARIANT["bufs_io"]) as rhsp,
            tc.tile_pool(name="blp", bufs=2) as blp,
            tc.tile_pool(name="midp", bufs=VARIANT["bufs_mid"]) as midp,
            tc.tile_pool(name="pp", bufs=VARIANT["bufs_pp"],
                         space="PSUM") as pp,
            tc.tile_pool(name="ep", bufs=VARIANT["bufs_ep"],
                         space="PSUM") as ep,
            tc.tile_pool(name="op", bufs=VARIANT["bufs_op"],
                         space="PSUM") as op,
        ):
            wt_sb = constp.tile([P, D], dt_ht)
            nc.sync.dma_start(wt_sb[:], wt_d[:])
            wabs_sb = constp.tile([P, 1], dt_act)
            nc.sync.dma_start(wabs_sb[:], wabs_d[:])
            bcol_sb = constp.tile([P, 1], f32)
            nc.sync.dma_start(bcol_sb[:], bcol_d[:])
            iota_sb = constp.tile([P, S], dt_rhs)
            nc.sync.dma_start(iota_sb[:], iota_d[:])
            negc_sb = constp.tile([P, 1], f32)
            nc.gpsimd.memset(negc_sb[:], -float(c_shift) + PRESCALE_LN)

            rhs_dma = getattr(nc, VARIANT["rhs_eng"])
            ht_dma = getattr(nc, VARIANT["ht_eng"])

            def emit_blocks():
              for i in range(NBLK):
                bl_sb = blp.tile([P, t_max], f32)
                nc.sync.dma_start(bl_sb[:], bl_d[i])
                out_ps = op.tile([S, D + 1], f32)
                for gd in range(NGD):
                    rhs_sb = rhsp.tile([P, DGRP, D + 1], dt_rhs)
                    rhs_dma.dma_start(
                        rhs_sb[:],
                        hrhs_d[i, gd].rearrange("p (t c) -> p t c", t=DGRP))
                    ht_sb = htp.tile([P, DGRP, P], dt_ht)
                    ht_dma.dma_start(
                        ht_sb[:],
                        ht_d[i, gd].rearrange("p (t c) -> p t c", t=DGRP))

                    for sub in range(DGRP // GRP):
                        sb = sub * GRP
                        # proj_T[h, node] = wt.T @ ht  (per tile)
                        proj_g = pp.tile([P, GRP, P], f32)
                        for tt in range(GRP):
                            nc.tensor.matmul(proj_g[:, tt], wt_sb[:],
                                             ht_sb[:, sb + tt],
                                             start=True, stop=True)
                        tanh_g = midp.tile([P, GRP, P], dt_act, tag="tanh")
                        nc.scalar.activation(
                            tanh_g[:], proj_g[:],
                            mybir.ActivationFunctionType.Tanh,
                            bias=bcol_sb[:], scale=act_scale)
                        # e[node] = |w| . tanh_T  (PE, contraction over h)
                        e_ps = ep.tile([P, GRP], f32)
                        for tt in range(GRP):
                            nc.tensor.matmul(e_ps[:, tt:tt + 1],
                                             tanh_g[:, tt], wabs_sb[:],
                                             start=True, stop=True)
                        a8 = midp.tile([P, GRP], f32, tag="a8")
                        nc.scalar.activation(
                            a8[:], e_ps[:], mybir.ActivationFunctionType.Exp,
                            bias=negc_sb[:])
                        for tt in range(GRP):
                            t = gd * DGRP + sb + tt
                            oh_sb = midp.tile([P, S], dt_rhs, tag="oh")
                            nc.vector.tensor_scalar(
                                oh_sb[:], iota_sb[:], bl_sb[:, t:t + 1],
                                a8[:, tt:tt + 1],
                                mybir.AluOpType.is_equal,
                                mybir.AluOpType.mult)
                            nc.tensor.matmul(out_ps[:], oh_sb[:],
                                             rhs_sb[:, sb + tt],
                                             start=(t == 0),
                                             stop=(t == t_max - 1))
                den_sb = midp.tile([S, 1], f32, tag="den")
                nc.vector.tensor_scalar_max(den_sb[:], out_ps[:, D:D + 1],
                                            1e-12)
                rec_sb = midp.tile([S, 1], f32, tag="rec")
                nc.vector.reciprocal(rec_sb[:], den_sb[:])
                res_sb = midp.tile([S, D], f32, tag="res")
                nc.vector.tensor_scalar(
                    res_sb[:], out_ps[:, 0:D], rec_sb[:], None,
                    mybir.AluOpType.mult)
                nc.sync.dma_start(out_d[i], res_sb[:])

            if repeat > 1:
                with tc.For_i(0, repeat, 1):
                    emit_blocks()
            else:
                emit_blocks()
    nc.compile()
    return nc


def _prep_inputs(H, batch, W_proj, b_proj, w_score, cfg):
    """Host-side repack. Returns (in_maps, t_max, c_shift, True)."""
    S = VARIANT["segs_per_blk"]
    DGRP = VARIANT["dma_grp"]
    NBLK_TOT = G_SEGS // S
    NBLK = NBLK_TOT // N_CORES
    V = H.shape[0]
    H = np.ascontiguousarray(H, dtype=np.float32)
    batch = np.asarray(batch).astype(np.int64)
    W_proj = np.asarray(W_proj, dtype=np.float32)
    b_proj = np.asarray(b_proj, dtype=np.float32)
    w_score = np.asarray(w_score, dtype=np.float32)

    c_shift = float(np.abs(w_score).sum())
    sgn = np.where(w_score >= 0.0, 1.0, -1.0).astype(np.float32)

    s = np.searchsorted(batch, np.arange(NBLK_TOT + 1, dtype=np.int64) * S)
    lens = s[1:] - s[:-1]
    t_max = int(math.ceil(lens.max() / P))
    t_max = max(DGRP, ((t_max + DGRP - 1) // DGRP) * DGRP)

    tpos = np.arange(t_max * P, dtype=np.int64)
    idx = s[:NBLK_TOT, None] + tpos[None, :]              # [NBLK_TOT, t_max*P]
    tile_active = (tpos[None, :] // P) * P < lens[:, None]
    valid = tile_active & (idx < V)
    idxc = np.minimum(idx, V - 1)

    blv = (batch[idxc] - (np.arange(NBLK_TOT, dtype=np.int64)[:, None]
                          * S)).astype(np.float32)
    blv[~valid] = -1000.0
    # bl layout: [blk, P(node-in-tile), t_max]
    bl = np.ascontiguousarray(
        blv.reshape(NBLK_TOT, t_max, P).transpose(0, 2, 1))

    NGD = t_max // DGRP
    dt_rhs = _np_dt(cfg["rhs"])
    dt_ht = _np_dt(cfg["ht"])

    # rhs = [H | 1] gathered; layout [blk, NGD, P(node), DGRP*(D+1)]
    Hg = H[idxc]                                          # [blk, t_max*P, D]
    Hg[~valid] = 0.0
    rhs = np.empty((NBLK_TOT, t_max * P, D + 1), dtype=np.float32)
    rhs[:, :, :D] = Hg
    rhs[:, :, D] = np.where(valid, 1.0, 0.0)
    rhs = rhs.reshape(NBLK_TOT, NGD, DGRP, P, D + 1).transpose(0, 1, 3, 2, 4)
    rhs = np.ascontiguousarray(rhs.reshape(NBLK_TOT, NGD, P, DGRP * (D + 1))
                               .astype(dt_rhs))

    # ht = H gathered, per-tile transposed;
    # layout [blk, NGD, P(feature d), DGRP*P(node)]
    if cfg["ht_scale"] != 1.0:
        Hg *= np.float32(cfg["ht_scale"])
    ht = Hg.reshape(NBLK_TOT, NGD, DGRP, P, D).transpose(0, 1, 4, 2, 3)
    ht = np.ascontiguousarray(ht.reshape(NBLK_TOT, NGD, P, DGRP * P)
                              .astype(dt_ht))
    del Hg

    wt = np.ascontiguousarray(
        ((W_proj * sgn[:, None]) * np.float32(cfg["wt_scale"])).T
        .astype(dt_ht))                                       # [d, h]
    wabs = np.ascontiguousarray(
        np.abs(w_score).reshape(P, 1).astype(_np_dt(cfg["act"])))
    bcol = np.ascontiguousarray((sgn * b_proj).reshape(P, 1)
                                .astype(np.float32))
    iota = np.ascontiguousarray(
        np.broadcast_to(np.arange(S, dtype=np.float32), (P, S))
        .astype(dt_rhs))

    in_maps = []
    for c in range(N_CORES):
        sl = slice(c * NBLK, (c + 1) * NBLK)
        in_maps.append({
            "hrhs": np.ascontiguousarray(rhs[sl]),
            "ht": np.ascontiguousarray(ht[sl]),
            "bl": np.ascontiguousarray(bl[sl]),
            "wt": wt,
            "wabs": wabs,
            "bcol": bcol,
            "iota": iota,
        })
    return in_maps, t_max, c_shift, True


def kernel(H, batch, W_proj, b_proj, w_score):
    from concourse.bass_utils import run_bass_kernel_spmd

    in_maps, t_max, c_shift, _ = _prep_inputs(
        H, batch, W_proj, b_proj, w_score, CFG)
    nc = _build_program(t_max, c_shift, CFG)
    res = run_bass_kernel_spmd(nc, in_maps, core_ids=list(range(N_CORES)))
    S = VARIANT["segs_per_blk"]
    NBLK = (G_SEGS // S) // N_CORES
    out = np.concatenate([r["out"].reshape(NBLK * S, D)
                          for r in res.results], axis=0)
    return out.astype(np.float32)
